# revision 10
# baseline (speedup 1.0000x reference)
"""Multi-head causal attention (B=2, T=2048, D=1024, H=16) on 8 Trainium2
NeuronCores.

Sharding: batch x head-group data/tensor parallel. Core c handles batch
c//4 and heads (c%4)*4 .. +4: W_qkv is split column-wise per head group,
W_o row-wise; each core computes attention for its local heads and a
partial output projection. The host sums the 4 partials per batch
(row-parallel W_o reduction) and stacks the two batches.

Per-core device kernel (fp16 data path, fp32 PSUM accumulate):
  - Q/K stored pair-packed [128, T]: head-even dims on partitions 0:64,
    head-odd on 64:128. QK^T for the two heads of a pair runs as two
    K=64 matmuls on disjoint PE row groups (tile_position (0,0)/(64,0)),
    which the PE executes concurrently -> the pair costs one matmul's
    wall time instead of two.
  - The pair's two score tiles land in one 2-bank PSUM tile ([:, 0:w]
    and [:, 512:512+w]) so a single ACT instruction computes exp for
    both heads (halves the ACT fixed-cost count; ACT is co-critical).
  - V tiles keep a per-head ones column so the AV matmul (M=65) also
    emits the softmax denominator row; normalization is deferred one
    pair: 1/den via DVE reciprocal, broadcast across partitions with a
    single K=2 matmul against a 2-row selector, then two DVE mults
    write normalized attn^T (fp16) for the W_o projection.
  - A burst of dummy matmuls on an SBUF ones tile at t=0 warms the PE
    HAM clock gate (cold K=4/8 halves the clock for the first ~3.4us of
    sustained activity) while the first DMAs land; input DMAs issue
    across all four DGE queues (SP/ACT/DVE/Pool) to cut the ~600ns/DMA
    serial issue cost on the critical path.
  - W_o projection in fp16, one [128,1024] output stage + single DMA
    per token tile; PSUM->SBUF copies split between DVE and GpSimd.
"""
import sys

for _p in ("/opt/trn_rl_repo", "/root/.axon_site/_ro/trn_rl_repo"):
    if _p not in sys.path:
        sys.path.insert(0, _p)

import numpy as np
import concourse.bass as bass
import concourse.mybir as mybir
import concourse.tile as tile
from concourse.vector_clock import ScopedClock
from concourse.bass_utils import run_bass_kernel_spmd

F32 = mybir.dt.float32
F16 = mybir.dt.float16
AF = mybir.ActivationFunctionType

B, T, D = 2, 2048, 1024
N_CORES = 8
HPC = 4            # heads per core
HL = HPC * 64      # 256 local head dims
NKT = T // 128     # 16 k-tiles per head
NQC = T // 512     # 4 q-chunks


class FixedTileContext(tile.TileContext):
    """Works around this walrus build's 1-sync-wait-per-instruction limit.

    1. `_add_instruction`: peel extra waits off any instruction onto
       standalone single-wait nops emitted just before it on the same
       engine (the sequencer executes them in order).
    2. `_drain_and_barrier`: replace the tail drain (which carries one wait
       per outstanding proc) with chained single-wait sync-engine nops
       followed by a wait-free drain.
    """

    def _add_instruction(self, inst):
        si = inst.sync_info
        if si is not None:
            waits = list(si.on_wait)
            if len(waits) > 1:
                eng = getattr(inst, "engine", None)
                eng_obj = self.nc.engines.get(eng) if eng is not None else None
                if eng_obj is not None:
                    for w in waits[:-1]:
                        nop = eng_obj.nop()
                        nop.ins.sync_info = mybir.SyncInfo(on_wait=[w], on_update=[])
                    inst.sync_info = mybir.SyncInfo(
                        on_wait=[waits[-1]], on_update=list(si.on_update)
                    )
        super()._add_instruction(inst)

    def _drain_and_barrier(self, tick_clock, wait_clock):
        vec = tick_clock.global_clock
        for proc in range(len(vec)):
            t = vec[proc]
            if t <= 0:
                continue
            partial = ScopedClock()
            partial.require_at_least(None, proc, t)
            w = self.nc.sync.nop()
            wait_clock.add_sem_waits(w.ins, partial)
        self.nc.sync.drain()
        self.nc.all_engine_barrier()
        assert self.sems is not None
        popped = self.nc._tile_sem_poison_stack.pop()
        assert popped is self._sem_poison
        self.nc.clear_and_free_semaphores(list(self.sems.allocated().values()))
        self.nc.all_engine_barrier()


def build_nc():
    nc = bass.Bass()
    # cx columns: [Wq01 128 | Wk01 128 | Wq23 128 | Wk23 128 | Wv 256 | xT 2048]
    cx = nc.declare_dram_parameter("cx", [D, 2816], F16, isOutput=False)
    wo = nc.declare_dram_parameter("wo", [HL, D], F16, isOutput=False)
    consts = nc.declare_dram_parameter("consts", [128, 128], F16, isOutput=False)
    out = nc.declare_dram_parameter("out", [T, D], F16, isOutput=True)

    with FixedTileContext(nc) as tc:
        with tc.tile_pool(name="persist", bufs=1) as pp, \
             tc.tile_pool(name="work", bufs=8) as wp, \
             tc.tile_pool(name="nwork", bufs=4) as nwp, \
             tc.tile_pool(name="psum", bufs=2, space="PSUM") as psp:
            ones_t = pp.tile([128, 64], F16, tag="ones")
            nc.gpsimd.memset(ones_t[:], 1.0)
            # K=33 broadcast selector: row 0 -> out rows 0:64, row 32 ->
            # 64:128 (partition bases must be 32-aligned). Rows 1-31 are
            # zero so the matching garbage rows of the rhs contribute
            # nothing; the den tiles are preset to 1.0 so their recip stays
            # finite (0 * inf/NaN would poison the product).
            sel_t = pp.tile([33, 128], F16, tag="sel")
            nc.gpsimd.memset(sel_t[:], 0.0)
            nc.gpsimd.memset(sel_t[0:1, 0:64], 1.0)
            nc.gpsimd.memset(sel_t[32:33, 64:128], 1.0)
            # persistent double-buffered denominator tiles (rows 0/32 hold
            # the two heads' denominators; other rows stay at the 1.0 preset)
            den_t = []
            for i in range(2):
                dn = pp.tile([33, 512], F16, tag=f"den{i}", name=f"den{i}")
                nc.gpsimd.memset(dn[:], 1.0)
                den_t.append(dn)

            # HAM warm-up: dummy accumulation chain on the ones tile keeps
            # the PE busy from t~0 so the clock gate opens (K=8/8) by the
            # time the first projection data lands.
            wu = psp.tile([128, 512], F32, tag="av", name="wu", bufs=4)
            for i in range(32):
                nc.tensor.matmul(
                    wu[0:64, 0:64], ones_t[:, 0:64], ones_t[:, 0:64],
                    start=(i == 0), stop=(i == 31),
                )

            comb = [pp.tile([128, 2816], F16, tag=f"comb{k}", name=f"comb{k}")
                    for k in range(8)]
            dma_eng = [nc.sync, nc.scalar, nc.gpsimd]
            # wave 1: pair-0 Q/K weights (first proj group's stationary data)
            for k in range(8):
                dma_eng[k % 3].dma_start(comb[k][:, 0:256], cx[k * 128:(k + 1) * 128, 0:256])
            # wave 2: x chunk 0 (first proj group's moving data)
            for k in range(8):
                dma_eng[k % 3].dma_start(comb[k][:, 768:1280], cx[k * 128:(k + 1) * 128, 768:1280])
            # V tiles keep a per-head ones column at offset 64; set once here
            # (the projection copy writes only cols 0:64 of each head slot).
            vp_t = [pp.tile([128, HPC * 65], F16, tag=f"v{i}", name=f"v{i}")
                    for i in range(NKT)]
            for i in range(NKT):
                v_ones = vp_t[i][:].rearrange("p (h c) -> p h c", c=65)[:, :, 64:65]
                nc.gpsimd.memset(v_ones, 1.0)
            # wave 3: remaining weights + mask + W_o
            for k in range(8):
                dma_eng[k % 3].dma_start(comb[k][:, 256:768], cx[k * 128:(k + 1) * 128, 256:768])
            consts_t = pp.tile([128, 128], F16, tag="consts")
            nc.sync.dma_start(consts_t[:], consts[:])
            wo_t = []
            for c in range(2):
                w = pp.tile([128, D], F16, tag=f"wo{c}", name=f"wo{c}")
                nc.scalar.dma_start(w[:], wo[c * 128:(c + 1) * 128, :])
                wo_t.append(w)
            # wave 4: x chunks 1-3
            for k in range(8):
                dma_eng[k % 3].dma_start(comb[k][:, 1280:2816], cx[k * 128:(k + 1) * 128, 1280:2816])

            # pair-packed per-pair Q and K tiles [128, T]: head-even dims on
            # partitions 0:64, head-odd on 64:128
            qp_t = [pp.tile([128, T], F16, tag=f"q{hp}", name=f"q{hp}") for hp in range(2)]
            kp_t = [pp.tile([128, T], F16, tag=f"k{hp}", name=f"k{hp}") for hp in range(2)]
            at_t = [pp.tile([128, T], F16, tag=f"at{c}", name=f"at{c}")
                    for c in range(2)]

            def proj_group(j, m):
                # m=0: Q pair0, m=1: K pair0, m=2: Q pair1, m=3: K pair1
                ps = psp.tile([128, 512], F32, tag="mm", name="ps_proj")
                for k in range(8):
                    nc.tensor.matmul(
                        ps[:],
                        comb[k][:, m * 128:(m + 1) * 128],
                        comb[k][:, 768 + j * 512:768 + (j + 1) * 512],
                        start=(k == 0), stop=(k == 7),
                    )
                cs = slice(j * 512, (j + 1) * 512)
                dst = qp_t[m // 2] if m % 2 == 0 else kp_t[m // 2]
                nc.vector.tensor_copy(dst[:, cs], ps[:])

            def v_tile(kt):
                ps = psp.tile([128, 256], F32, tag="mm", name="ps_v")
                for k in range(8):
                    nc.tensor.matmul(
                        ps[:],
                        comb[k][:, 768 + kt * 128:768 + (kt + 1) * 128],
                        comb[k][:, 512:768],
                        start=(k == 0), stop=(k == 7),
                    )
                vt = vp_t[kt]
                v_view = vt[:].rearrange("p (h c) -> p h c", c=65)
                ps_view = ps[:].rearrange("p (h c) -> p h c", c=64)
                nc.vector.tensor_copy(v_view[:, :, 0:64], ps_view[:])

            pending_norm = []

            def flush_norm():
                while pending_norm:
                    pending_norm.pop(0)()

            def attn_pair(j, hp):
                """Attention for head pair (2hp, 2hp+1) at chunk j. The two
                heads' QK matmuls run concurrently on disjoint PE row groups;
                one ACT exp covers both heads. AV chains (M=65, with the
                denominator row from the V ones column) accumulate per head.
                Queues a deferred normalization closure so its recip/bcast
                latency hides behind the next pair's matmuls."""
                av_e = psp.tile([65, 512], F32, tag="av", name="av_e", bufs=4)
                av_o = psp.tile([65, 512], F32, tag="av", name="av_o", bufs=4)
                nkt = 4 * j + 4
                qcs = slice(j * 512, (j + 1) * 512)

                def score(kt):
                    """Row-tiled QK pair + batched exp for one k-tile.
                    Diagonal-crossing tiles (kt >= 4j) are column-restricted
                    to their causally nonzero range [d4*128, 512); only the
                    first 128 columns of that range are triangular and get
                    the mask multiply. Returns per-head AV operand lists of
                    (expS_slice, out_col_offset, width)."""
                    d4 = kt - 4 * j
                    if d4 < 0:
                        c0, w = 0, 512
                    else:
                        c0, w = d4 * 128, 512 - d4 * 128
                    ks = slice(kt * 128, (kt + 1) * 128)
                    qs = slice(j * 512 + c0, (j + 1) * 512)
                    sp = psp.tile([128, 1024], F32, tag="mm", name="sp")
                    nc.tensor.matmul(
                        sp[:, 0:w], kp_t[hp][0:64, ks], qp_t[hp][0:64, qs],
                        start=True, stop=True,
                    )
                    nc.tensor.matmul(
                        sp[:, 512:512 + w], kp_t[hp][64:128, ks], qp_t[hp][64:128, qs],
                        start=True, stop=True,
                    )
                    et = wp.tile([128, 1024], F16, tag="e", name="et")
                    sp3 = sp[:].rearrange("p (g c) -> p g c", c=512)[:, :, 0:w]
                    et3 = et[:].rearrange("p (g c) -> p g c", c=512)[:, :, 0:w]
                    nc.scalar.activation(et3, sp3, AF.Exp, scale=0.125)
                    if d4 < 0:
                        return [[(et[:, 0:512], 0, 512)],
                                [(et[:, 512:1024], 0, 512)]]
                    emt = wp.tile([128, 256], F16, tag="em", name="emt")
                    parts = []
                    for g in range(2):
                        nc.vector.tensor_mul(
                            emt[:, g * 128:(g + 1) * 128],
                            et[:, g * 512:g * 512 + 128], consts_t[:],
                        )
                        p = [(emt[:, g * 128:(g + 1) * 128], c0, 128)]
                        if w > 128:
                            p.append((et[:, g * 512 + 128:g * 512 + w], c0 + 128, w - 128))
                        parts.append(p)
                    return parts

                # stagger: QK(kt+1) issues before AV(kt) so AV's wait on the
                # fresh expS tile is already satisfied at queue head.
                srcs = {0: score(0)}
                for kt in range(nkt):
                    if kt + 1 < nkt:
                        srcs[kt + 1] = score(kt + 1)
                    parts_e, parts_o = srcs.pop(kt)
                    for g, (av, parts) in enumerate(((av_e, parts_e), (av_o, parts_o))):
                        h = 2 * hp + g
                        for pi, (src, c0, w) in enumerate(parts):
                            nc.tensor.matmul(
                                av[:, c0:c0 + w],
                                vp_t[kt][:, h * 65:(h + 1) * 65],
                                src,
                                start=(kt == 0),
                                stop=(kt == nkt - 1 and pi == len(parts) - 1),
                                skip_group_check=True,
                            )
                den = den_t[(2 * j + hp) % 2]
                nc.vector.tensor_copy(den[0:1, :], av_e[64:65, :])
                nc.vector.tensor_copy(den[32:33, :], av_o[64:65, :])

                def norm():
                    rec = nwp.tile([33, 512], F16, tag="rec", name="rec")
                    with nc.allow_low_precision(reason="softmax recip"):
                        nc.vector.reciprocal(rec[:], den[:])
                    bc = psp.tile([128, 512], F32, tag="mm", name="bc")
                    nc.tensor.matmul(bc[:], sel_t[0:33, 0:128], rec[0:33, :],
                                     start=True, stop=True)
                    bcs = nwp.tile([128, 512], F16, tag="bcs", name="bcs")
                    nc.vector.tensor_copy(bcs[:], bc[:])
                    with nc.allow_low_precision(reason="normalized attn"):
                        nc.vector.tensor_mul(
                            at_t[hp][0:64, qcs], av_e[0:64, :], bcs[0:64, :])
                        nc.vector.tensor_mul(
                            at_t[hp][64:128, qcs], av_o[0:64, :], bcs[64:128, :])

                pending_norm.append(norm)

            def wo_chunk(j, tail=False):
                # out rows for q-chunk j; needs attnT[:, j-chunk] (both pairs
                # of chunk j normalized).
                for t in range(4 * j, 4 * j + 4):
                    os = nwp.tile([128, D], F16, tag="os", name="os")
                    for n in range(2):
                        wpb = psp.tile([128, 512], F32, tag="mm", name="wpb")
                        for c in range(2):
                            nc.tensor.matmul(
                                wpb[:],
                                at_t[c][:, t * 128:(t + 1) * 128],
                                wo_t[c][:, n * 512:(n + 1) * 512],
                                start=(c == 0), stop=(c == 1),
                            )
                        cs = slice(n * 512, (n + 1) * 512)
                        if tail and n == 1:
                            # ACT is idle in the kernel tail
                            nc.scalar.copy(os[:, cs], wpb[:])
                        else:
                            nc.vector.tensor_copy(os[:, cs], wpb[:])
                    (nc.sync if t % 2 == 0 else nc.scalar).dma_start(
                        out[t * 128:(t + 1) * 128, :], os[:])

            for j in range(NQC):
                proj_group(j, 0)
                proj_group(j, 1)
                for kt in range(4 * j, 4 * j + 4):
                    v_tile(kt)
                attn_pair(j, 0)
                while len(pending_norm) > 1:
                    pending_norm.pop(0)()
                if 0 < j < NQC - 1:
                    wo_chunk(j - 1)
                proj_group(j, 2)
                proj_group(j, 3)
                attn_pair(j, 1)
                while len(pending_norm) > 1:
                    pending_norm.pop(0)()
            # tail: issue the last pair's norm first so its DVE chain runs
            # while wo(2)'s matmuls keep the PE busy, then wo(3) follows.
            flush_norm()
            wo_chunk(NQC - 2)
            wo_chunk(NQC - 1, tail=True)
    return nc


def _make_masks():
    p = np.arange(128)[:, None]
    f = np.arange(128)[None, :]
    return (p <= f).astype(np.float16)


_NC_CACHE = {}


def make_in_maps(x, W_qkv, W_o):
    x = np.ascontiguousarray(np.asarray(x, dtype=np.float32))
    W_qkv = np.ascontiguousarray(np.asarray(W_qkv, dtype=np.float32))
    W_o = np.ascontiguousarray(np.asarray(W_o, dtype=np.float32))
    W_q, W_k, W_v = W_qkv[:, :D], W_qkv[:, D:2 * D], W_qkv[:, 2 * D:]
    masks = _make_masks()

    in_maps = []
    for c in range(N_CORES):
        b, g = c // 4, c % 4
        cols = slice(g * HL, (g + 1) * HL)
        wq, wk, wv = W_q[:, cols], W_k[:, cols], W_v[:, cols]
        cxv = np.concatenate(
            [wq[:, 0:128], wk[:, 0:128], wq[:, 128:256], wk[:, 128:256],
             wv, x[b].T], axis=1
        ).astype(np.float16)
        in_maps.append({
            "cx": np.ascontiguousarray(cxv),
            "wo": np.ascontiguousarray(W_o[g * HL:(g + 1) * HL, :]).astype(np.float16),
            "consts": masks,
        })
    return in_maps


def kernel(x, W_qkv, W_o):
    if "nc" not in _NC_CACHE:
        _NC_CACHE["nc"] = build_nc()
    nc = _NC_CACHE["nc"]

    in_maps = make_in_maps(x, W_qkv, W_o)
    res = run_bass_kernel_spmd(nc, in_maps, list(range(N_CORES)))
    out = np.zeros((B, T, D), dtype=np.float32)
    for c in range(N_CORES):
        out[c // 4] += res.results[c]["out"].astype(np.float32)
    return out
